# revision 14
# baseline (speedup 1.0000x reference)
"""Trainium2 Bass kernel for GNN message passing (nn_Brain) — v3.

Reference semantics (per batch b, 20 steps):
    act = zeros(100000); act[:1024] = x_b
    repeat 20: act += tanh(segment_sum(act[from_idx]*w, to_idx) + bias);
               act[:1024] = x_b
    out_b = act[-1024:]

Mapping onto 8 NeuronCores (dest-sharded, batch across partitions):
  * NC r owns dests [r*12500, (r+1)*12500); Q7 core k gathers from source
    chunk k; SBUF partition 16k+b holds chunk k's act for batch b.
  * Per (core, tile): ap_gather acts; mul by int16-held weights; in-place
    cumsum; ap_gather at per-dest segment ends; adjacent diff -> per-core
    partials; [128,8] PE matmul sums the 8 cores per batch; DMA to a DRAM
    total buffer; epilogue adds bias, applies the input clamp via cmask,
    tanh, accumulates, and an AllGather + table-refresh DMAs publish the
    new act slices.

Perf structure (wall ~= 95ms dispatch + ~790ms input upload over the
axon tunnel at ~19ms/MB + ~185ms device exec; measured per-op costs:
ap_gather ~30-45ns/idx on GPSIMD which is the bottleneck engine,
AllGather ~1.1ms, DVE ops ~4-25us):
  * inputs are entropy-packed: 14-bit source indices and 14-bit
    quantized weights, both 8-values-in-7-int16-lanes (weight unpack on
    device, interleaved into step 0 where it hides under the gathers;
    int14 weights give rel err ~8.8e-3 vs the 2e-2 gate — int8 fails at
    0.56 because the 20-step dynamics amplify weight error ~70x).
  * software-pipelined emission: gather(t+1) queues on GPSIMD before
    extract(t), so DVE work (mul+scan) hides under the next gather and
    GPSIMD runs back-to-back; gp/ep/wp pools double-buffered.
  * ones-free in-place cumsum: tensor_tensor_scan(g, g, g, 0, add,
    bypass) (bypass keeps arg0) — saves a [P,T] ones tile.
  * PSUM->SBUF moves on DVE (ScalarE sync hop measured ~40us vs ~5us).
  * epilogue: tot += bias; tot *= cmask; tanh in place; aslice += tot
    (clamped dests get tanh(0)=0 forever; aslice starts at the clamp x).
  * the final step skips AllGather + refresh (nothing consumes them).
"""

import jax
jax.config.update("jax_compilation_cache_dir", "/tmp/jaxcache")
jax.config.update("jax_persistent_cache_min_compile_time_secs", 0)
jax.config.update("jax_persistent_cache_min_entry_size_bytes", 0)

import numpy as np
from contextlib import ExitStack

import concourse.bacc as bacc
import concourse.mybir as mybir
from concourse.tile import TileContext
from concourse import bass_utils
import bass_rust as _bass_rust

def _dep(a, b, reason):
    _bass_rust.add_dep_helper(a.ins, b.ins, True, reason)

F32 = mybir.dt.float32
BF16 = mybir.dt.bfloat16
I16 = mybir.dt.int16

STEPS = 20
IN_SIZE = 1024
OUT_SIZE = 1024
N = 100000
B = 8
NCD = 8
NK = 8
CH = N // NCD
T = 8448
DPX = 704
DMAX = DPX - 1
SLICE_PAD = 12544
PB = SLICE_PAD // 128  # 98
P = 128
STRIP = 16
MM8 = True
SCAN_BYPASS = True
REFRESH3D = False


def _wrap_stream(a):
    NKd, NT, L = a.shape
    aw = a.reshape(NKd, NT, L // 16, 16).transpose(0, 3, 1, 2)
    return np.ascontiguousarray(aw.reshape(NKd * 16, NT * (L // 16)))


def _preprocess(x, w, bias, from_idx, to_idx):
    E = from_idx.shape[0]
    r_arr = (to_idx // CH).astype(np.int32)
    k_arr = (from_idx // CH).astype(np.int32)
    ld = (to_idx % CH).astype(np.int32)
    ls = (from_idx % CH).astype(np.int16)
    strm = r_arr * NK + k_arr
    key = strm.astype(np.int64) * CH + ld
    cnt = np.bincount(key, minlength=64 * CH).reshape(64, CH)
    ccnt = cnt.cumsum(axis=1)

    bounds = []
    s = 0
    base = np.zeros(64, np.int64)
    while s < CH:
        hi = min(s + DMAX, CH)
        if (ccnt[:, hi - 1] - base).max() <= T - 1:
            e = hi
        else:
            lo = s + 1
            h2 = hi
            while lo < h2:
                mid = (lo + h2 + 1) // 2
                if (ccnt[:, mid - 1] - base).max() <= T - 1:
                    lo = mid
                else:
                    h2 = mid - 1
            e = lo
        assert e > s
        bounds.append((s, e))
        base = ccnt[:, e - 1].astype(np.int64).copy()
        s = e
    NT = len(bounds)
    ends = np.array([b[1] for b in bounds])

    tile_of = np.searchsorted(ends, ld, side="right").astype(np.int32)
    # innermost key ls: edges sorted by src within each dest group
    # (order-invariant for the sum; measured ~7% faster ap_gather)
    order = np.lexsort((ls, ld, tile_of, strm))
    so_strm = strm[order]
    so_tile = tile_of[order]
    gkey = so_strm.astype(np.int64) * NT + so_tile
    newg = np.empty(E, bool)
    newg[0] = True
    newg[1:] = gkey[1:] != gkey[:-1]
    gstart = np.flatnonzero(newg)
    gid = np.cumsum(newg) - 1
    pos = np.arange(E, dtype=np.int64) - gstart[gid] + 1
    assert pos.max() <= T - 1

    idx_stream = np.zeros((64, NT, T), np.int16)
    w_stream = np.zeros((64, NT, T), np.int16)
    idx_stream[so_strm, so_tile, pos] = ls[order]
    # 14-bit weight quantization (rel err ~8.8e-3 after 20 chaotic steps;
    # int13 would land at 1.9e-2, over the 2e-2 gate)
    wscale = float(np.abs(w).max()) / 8191.0
    w_stream[so_strm, so_tile, pos] = np.round(w[order] / wscale).astype(np.int16)

    eidx = np.zeros((64, NT, DPX), np.int16)
    for tix, (s0, e0) in enumerate(bounds):
        base_t = ccnt[:, s0 - 1] if s0 > 0 else np.zeros(64, np.int64)
        vals = ccnt[:, s0:e0] - np.asarray(base_t)[:, None]
        eidx[:, tix, 1:1 + (e0 - s0)] = vals.astype(np.int16)

    # Stationary matrix [P, B]: sums the 8 per-core partials of batch b
    # into PSUM partition b; entries are wscale (undo int16 weight quant).
    mmat = np.zeros((P, B if MM8 else P), np.float32)
    for p in range(P):
        if p % 16 < 8:
            mmat[p, p % 16] = wscale

    in_maps = []
    for r in range(NCD):
        sl = slice(r * NK, (r + 1) * NK)
        idx_w = _wrap_stream(idx_stream[sl])
        iw = idx_w.astype(np.uint16).reshape(P, NT, T // 16 // 8, 8)
        lv = np.zeros((P, NT, T // 16 // 8, 7), np.uint16)
        lv[..., 0] = iw[..., 0] | (iw[..., 1] << 14)
        lv[..., 1] = (iw[..., 1] >> 2) | (iw[..., 2] << 12)
        lv[..., 2] = (iw[..., 2] >> 4) | (iw[..., 3] << 10)
        lv[..., 3] = (iw[..., 3] >> 6) | (iw[..., 4] << 8)
        lv[..., 4] = (iw[..., 4] >> 8) | (iw[..., 5] << 6)
        lv[..., 5] = (iw[..., 5] >> 10) | (iw[..., 6] << 4)
        lv[..., 6] = (iw[..., 6] >> 12) | (iw[..., 7] << 2)
        idx_pk = np.ascontiguousarray(
            lv.reshape(P, NT * (T // 16 // 8) * 7)).view(np.int16)
        eidx_w = _wrap_stream(eidx[sl])
        # weights: pack 8 consecutive 14-bit fields into 7 int16 lanes,
        # row-major per core stream (unpacked on device in the prologue)
        wf = (w_stream[sl].reshape(NK, NT * T).astype(np.int64)
              & 0x3FFF).astype(np.uint16).reshape(NK, NT * T // 8, 8)
        wl = np.zeros((NK, NT * T // 8, 7), np.uint16)
        wl[..., 0] = wf[..., 0] | (wf[..., 1] << 14)
        wl[..., 1] = (wf[..., 1] >> 2) | (wf[..., 2] << 12)
        wl[..., 2] = (wf[..., 2] >> 4) | (wf[..., 3] << 10)
        wl[..., 3] = (wf[..., 3] >> 6) | (wf[..., 4] << 8)
        wl[..., 4] = (wf[..., 4] >> 8) | (wf[..., 5] << 6)
        wl[..., 5] = (wf[..., 5] >> 10) | (wf[..., 6] << 4)
        wl[..., 6] = (wf[..., 6] >> 12) | (wf[..., 7] << 2)
        w_hbm = np.ascontiguousarray(
            wl.reshape(NK, NT * T // 8 * 7)).view(np.int16)

        bias_t = np.zeros((P, PB), np.float32)
        for part in range(P):
            l0 = part * PB
            lend = min(l0 + PB, CH)
            if lend > l0:
                bias_t[part, 0:lend - l0] = bias[r * CH + l0:r * CH + lend]

        cmask = np.ones((STRIP, B * PB), np.float32)
        cx = np.zeros((STRIP, B * PB), np.float32)
        if r == 0:
            for part in range(STRIP):
                l0 = part * PB
                ncl = min(IN_SIZE - l0, PB)
                if ncl <= 0:
                    continue
                for b in range(B):
                    cmask[part, b * PB:b * PB + ncl] = 0.0
                    cx[part, b * PB:b * PB + ncl] = x[b, l0:l0 + ncl]
        in_maps.append(dict(
            idxs=idx_pk, eidxs=eidx_w, whbm=w_hbm, xin=x.astype(np.float32),
            biast=bias_t, cmask=cmask, cx=cx, mmat=mmat,
        ))
    dts = [(b[1] - b[0]) for b in bounds]
    offs = [b[0] for b in bounds]
    return in_maps, NT, dts, offs


def _build(NT, dts, offs, steps, scan_bypass=True, mm8=True, refresh3d=False, ab=frozenset()):
    nc = bacc.Bacc("TRN2", target_bir_lowering=False, debug=False,
                   num_devices=NCD)

    PKL = (T // 16 // 8) * 7
    idx_d = nc.dram_tensor("idxs", [P, NT * PKL], I16, kind="ExternalInput")
    eidx_d = nc.dram_tensor("eidxs", [P, NT * (DPX // 16)], I16, kind="ExternalInput")
    PKW = (T // 8) * 7
    w_d = nc.dram_tensor("whbm", [NK, NT * PKW], I16, kind="ExternalInput")
    w_s = nc.dram_tensor("w_scratch", [NK, NT * T], I16)
    x_d = nc.dram_tensor("xin", [B, IN_SIZE], F32, kind="ExternalInput")
    bias_d = nc.dram_tensor("biast", [P, PB], F32, kind="ExternalInput")
    cmask_d = nc.dram_tensor("cmask", [STRIP, B * PB], F32, kind="ExternalInput")
    cx_d = nc.dram_tensor("cx", [STRIP, B * PB], F32, kind="ExternalInput")
    mmat_d = nc.dram_tensor("mmat", [P, B if mm8 else P], F32, kind="ExternalInput")

    total_d = nc.dram_tensor("total_dram", [B, SLICE_PAD], F32)
    ag_in = nc.dram_tensor("ag_in", [B, SLICE_PAD], F32)
    ag_out = nc.dram_tensor("ag_out", [NCD * B, SLICE_PAD], F32,
                            addr_space="Shared")
    out_d = nc.dram_tensor("out", [B, OUT_SIZE], F32, kind="ExternalOutput")

    ADD, BYP = mybir.AluOpType.add, mybir.AluOpType.bypass

    with TileContext(nc) as tc, ExitStack() as ctx:
        cpool = ctx.enter_context(tc.tile_pool(name="const", bufs=1))
        wp = ctx.enter_context(tc.tile_pool(name="wp", bufs=2))
        gp = ctx.enter_context(tc.tile_pool(name="gp", bufs=2))
        ep = ctx.enter_context(tc.tile_pool(name="ep", bufs=2))
        dp = ctx.enter_context(tc.tile_pool(name="dp", bufs=2))
        pp = ctx.enter_context(tc.tile_pool(name="pp", bufs=2, space="PSUM"))
        pkp = ctx.enter_context(tc.tile_pool(name="pkp", bufs=2))
        tmpp = ctx.enter_context(tc.tile_pool(name="tmpp", bufs=2))
        sp = ctx.enter_context(tc.tile_pool(name="sp", bufs=2))
        slp = ctx.enter_context(tc.tile_pool(name="slp", bufs=1))

        # Resident data
        table_t = cpool.tile([P, CH], F32)
        nc.vector.memset(table_t[:], 0.0)
        nc.sync.dma_start(table_t[0:B, 0:IN_SIZE], x_d[:])
        mmat_t = cpool.tile([P, B if mm8 else P], F32)
        nc.sync.dma_start(mmat_t[:], mmat_d[:])
        eidx_t = cpool.tile([P, NT * (DPX // 16)], I16)
        nc.sync.dma_start(eidx_t[:], eidx_d[:])
        idx_t = cpool.tile([P, NT * (T // 16)], I16)
        zb_t = cpool.tile([P, 1], F32)   # dummy data1 for the bypass scan
        nc.vector.memset(zb_t[:], 0.0)
        zb_bc = zb_t[:].broadcast_to((P, T))
        ones_t = None
        if not scan_bypass:
            ones_t = cpool.tile([P, T], BF16)
            nc.vector.memset(ones_t[:], 1.0)
        NG = T // 16 // 8
        AND, SHR, SHL, OR = (mybir.AluOpType.bitwise_and,
                             mybir.AluOpType.logical_shift_right,
                             mybir.AluOpType.logical_shift_left,
                             mybir.AluOpType.bitwise_or)
        for t in range(NT):
            pk_t = pkp.tile([P, PKL], I16, tag="pk")
            nc.sync.dma_start(pk_t[:], idx_d[:, t * PKL:(t + 1) * PKL])
            pkv = pk_t[:].rearrange("p (g l) -> p g l", l=7)
            ov = idx_t[:, t * (T // 16):(t + 1) * (T // 16)].rearrange(
                "p (g e) -> p g e", e=8)
            nc.vector.tensor_single_scalar(
                ov[:, :, 0:1], pkv[:, :, 0:1], 0x3FFF, AND)
            nc.vector.tensor_scalar(
                ov[:, :, 7:8], pkv[:, :, 6:7], 2, 0x3FFF, SHR, AND)
            for o in range(1, 7):
                tmp_t = tmpp.tile([P, NG], I16, tag="tmp")
                tm2_t = tmpp.tile([P, NG], I16, tag="tm2")
                tv = tmp_t[:].rearrange("p (g o) -> p g o", o=1)
                tv2 = tm2_t[:].rearrange("p (g o) -> p g o", o=1)
                nc.vector.tensor_scalar(
                    tv, pkv[:, :, o - 1:o], 16 - 2 * o, (1 << (2 * o)) - 1,
                    SHR, AND)
                nc.vector.tensor_single_scalar(
                    tv2, pkv[:, :, o:o + 1], 2 * o, SHL)
                nc.vector.tensor_tensor(tv, tv, tv2, OR)
                nc.vector.tensor_single_scalar(
                    ov[:, :, o:o + 1], tv, 0x3FFF, AND)

        # 14-bit weight unpack (emitted per tile inside step 0, where the
        # ~200us of DVE work per tile hides under the ~293us GPSIMD gather).
        # Reuses wp-pool tiles plus one small tmp tag; sign extension via
        # mask/xor/sub, correct for a 32-bit sign-extending ALU.
        XOR, SUB = mybir.AluOpType.bitwise_xor, mybir.AluOpType.subtract
        GW = T // 8

        def emit_w_unpack(t):
            wq_t = wp.tile([P, T], I16, tag="w")
            nc.sync.dma_start(wq_t[0:NK, 0:PKW], w_d[:, t * PKW:(t + 1) * PKW])
            wu_t = wp.tile([P, T], I16, tag="w")
            qv = wq_t[0:NK, 0:PKW].rearrange("p (g l) -> p g l", l=7)
            uv = wu_t[0:NK, :].rearrange("p (g e) -> p g e", e=8)
            nc.vector.tensor_scalar(
                uv[:, :, 0:1], qv[:, :, 0:1], 0x3FFF, 0x2000, AND, XOR)
            nc.vector.tensor_single_scalar(
                uv[:, :, 0:1], uv[:, :, 0:1], 0x2000, SUB)
            nc.vector.tensor_scalar(
                uv[:, :, 7:8], qv[:, :, 6:7], 2, 0x3FFF, SHR, AND)
            nc.vector.tensor_single_scalar(
                uv[:, :, 7:8], uv[:, :, 7:8], 0x2000, XOR)
            nc.vector.tensor_single_scalar(
                uv[:, :, 7:8], uv[:, :, 7:8], 0x2000, SUB)
            for o in range(1, 7):
                wt_t = tmpp.tile([P, GW], I16, tag="wt")
                wv = wt_t[0:NK, :].rearrange("p (g o) -> p g o", o=1)
                nc.vector.tensor_scalar(
                    wv, qv[:, :, o - 1:o], 16 - 2 * o, (1 << (2 * o)) - 1,
                    SHR, AND)
                nc.vector.tensor_single_scalar(
                    uv[:, :, o:o + 1], qv[:, :, o:o + 1], 2 * o, SHL)
                nc.vector.tensor_tensor(uv[:, :, o:o + 1], uv[:, :, o:o + 1],
                                        wv, OR)
                nc.vector.tensor_scalar(
                    uv[:, :, o:o + 1], uv[:, :, o:o + 1], 0x3FFF, 0x2000,
                    AND, XOR)
                nc.vector.tensor_single_scalar(
                    uv[:, :, o:o + 1], uv[:, :, o:o + 1], 0x2000, SUB)
            return nc.sync.dma_start(w_s[:, t * T:(t + 1) * T], wu_t[0:NK, :])

        cmask_t = slp.tile([P, B * PB], F32)
        nc.vector.memset(cmask_t[:], 1.0)
        nc.sync.dma_start(cmask_t[0:STRIP, :], cmask_d[:])
        aslice_t = slp.tile([P, B * PB], F32)
        nc.vector.memset(aslice_t[:], 0.0)
        nc.sync.dma_start(aslice_t[0:STRIP, :], cx_d[:])
        bias_s = slp.tile([P, PB], F32)
        nc.sync.dma_start(bias_s[:], bias_d[:])
        bias_f = slp.tile([P, B * PB], F32)
        for b in range(B):
            nc.vector.tensor_copy(bias_f[:, b * PB:(b + 1) * PB], bias_s[:])

        prev_state = {"readback": None, "collective": None}

        def step_body(step):
            out_dmas = []
            cur = {}
            order2 = "serialext" in ab
            for t in range(NT + 1):
                if order2 and t == NT:
                    break
                if t < NT:
                    if step == 0:
                        wu_dma = emit_w_unpack(t)
                    w_t = wp.tile([P, T], I16, tag="w")
                    if "smallwdma" in ab:
                        w_src = w_s[:, t * T:t * T + 64].rearrange(
                            "k (o t) -> k o t", o=1).broadcast_to((NK, 16, 64))
                        wdma = nc.sync.dma_start(w_t[:, 0:64], w_src)
                    else:
                        w_src = w_s[:, t * T:(t + 1) * T].rearrange(
                            "k (o t) -> k o t", o=1).broadcast_to((NK, 16, T))
                        wdma = nc.sync.dma_start(w_t[:], w_src)
                    if step == 0:
                        _dep(wdma, wu_dma, "RAW w_scratch")
                    g_t = gp.tile([P, T], F32, tag="g")
                    if "nogather" in ab:
                        nc.vector.memset(g_t[:], 0.001)
                    else:
                        nc.gpsimd.ap_gather(
                            g_t[:], table_t[:],
                            idx_t[:, t * (T // 16):(t + 1) * (T // 16)],
                            channels=P, num_elems=CH, d=1, num_idxs=T)
                if (t >= 1 and not order2) or (order2 and False):
                    tp = t - 1
                    extr_t = ep.tile([P, DPX], F32, tag="extr")
                    if "noextract" in ab:
                        nc.vector.memset(extr_t[:], 0.001)
                    else:
                        nc.gpsimd.ap_gather(
                            extr_t[:], cur["g"],
                            eidx_t[:, tp * (DPX // 16):(tp + 1) * (DPX // 16)],
                            channels=P, num_elems=T, d=1, num_idxs=DPX)
                if t < NT:
                    if "nomul" not in ab:
                        nc.vector.tensor_mul(g_t[:], g_t[:], w_t[:])
                    if "noscan" in ab:
                        pass
                    elif scan_bypass:
                        nc.vector.tensor_tensor_scan(
                            g_t[:], g_t[:], zb_bc, 0.0, ADD, BYP)
                    else:
                        nc.vector.tensor_tensor_scan(
                            g_t[:], ones_t[:], g_t[:], 0.0,
                            mybir.AluOpType.mult, mybir.AluOpType.add)
                if order2 and t < NT:
                    tp = t
                    extr_t = ep.tile([P, DPX], F32, tag="extr")
                    if "noextract" in ab:
                        nc.vector.memset(extr_t[:], 0.001)
                    else:
                        nc.gpsimd.ap_gather(
                            extr_t[:], g_t[:],
                            eidx_t[:, tp * (DPX // 16):(tp + 1) * (DPX // 16)],
                            channels=P, num_elems=T, d=1, num_idxs=DPX)
                if (t >= 1 and not order2) or (order2 and t < NT):
                    if not order2:
                        tp = t - 1
                    diff_t = dp.tile([P, DPX - 1], F32, tag="diff")
                    nc.vector.tensor_sub(diff_t[:], extr_t[:, 1:DPX],
                                         extr_t[:, 0:DPX - 1])
                    ps_t = pp.tile([B if mm8 else P, DPX - 1], F32, tag="ps")
                    nc.tensor.matmul(ps_t[:, 0:512], mmat_t[:],
                                     diff_t[:, 0:512], start=True, stop=True)
                    if dts[tp] > 512:
                        nc.tensor.matmul(ps_t[:, 512:DPX - 1], mmat_t[:],
                                         diff_t[:, 512:DPX - 1],
                                         start=True, stop=True)
                    st_t = sp.tile([B, DPX - 1], F32, tag="st")
                    nc.vector.tensor_copy(st_t[:, 0:dts[tp]],
                                          ps_t[0:B, 0:dts[tp]])
                    od = nc.sync.dma_start(
                        total_d[:, offs[tp]:offs[tp] + dts[tp]],
                        st_t[:, 0:dts[tp]])
                    out_dmas.append(od)
                    if prev_state["readback"] is not None:
                        _dep(od, prev_state["readback"],
                             "WAR total_d across steps")
                if t < NT:
                    cur = {"g": g_t[:]}

            # Epilogue
            if "noepi" in ab:
                prev_state["readback"] = None
                return
            tot_t = slp.tile([P, B * PB], F32, tag="tot")
            rb = nc.sync.dma_start(
                tot_t[:].rearrange("p (b c) -> p b c", b=B),
                total_d[:].rearrange("b (p c) -> p b c", p=P))
            for od in out_dmas:
                _dep(rb, od, "RAW total_d")
            nc.vector.tensor_add(tot_t[:], tot_t[:], bias_f[:])
            nc.vector.tensor_mul(tot_t[:], tot_t[:], cmask_t[:])
            nc.scalar.activation(tot_t[:], tot_t[:],
                                 mybir.ActivationFunctionType.Tanh)
            nc.vector.tensor_add(aslice_t[:], aslice_t[:], tot_t[:])
            prev_state["readback"] = rb
            if step == steps - 1:
                return  # nothing consumes the collective after the last step
            if "nocc" in ab:
                return
            wb = nc.sync.dma_start(
                ag_in[:].rearrange("b (p c) -> p b c", p=P),
                aslice_t[:].rearrange("p (b c) -> p b c", b=B))
            if prev_state["collective"] is not None:
                _dep(wb, prev_state["collective"], "WAR ag_in")
            cc = nc.gpsimd.collective_compute(
                "AllGather", mybir.AluOpType.bypass,
                replica_groups=[list(range(NCD))],
                ins=[ag_in[:]], outs=[ag_out[:]])
            _dep(cc, wb, "RAW ag_in")
            if refresh3d:
                tr = nc.sync.dma_start(
                    table_t[:].rearrange("(k s) c -> k s c", s=16)[:, 0:B, :],
                    ag_out[:].rearrange("(k b) c -> k b c", k=NK)[:, :, 0:CH])
                _dep(tr, cc, "RAW ag_out")
            else:
                for k in range(NK):
                    tr = nc.sync.dma_start(
                        table_t[16 * k:16 * k + B, :],
                        ag_out[B * k:B * (k + 1), 0:CH])
                    _dep(tr, cc, "RAW ag_out")
            prev_state["collective"] = cc

        for s in range(steps):
            step_body(s)

        # Final output from aslice_t (tail0 = 11476 = 117*98 + 10).
        nc.sync.dma_start(
            out_d[:, 0:88].rearrange("b (o c) -> o b c", o=1),
            aslice_t[117:118, :].rearrange("p (b c) -> p b c", b=B)[:, :, 10:98])
        nc.sync.dma_start(
            out_d[:, 88:970].rearrange("b (p c) -> p b c", p=9),
            aslice_t[118:127, :].rearrange("p (b c) -> p b c", b=B))
        nc.sync.dma_start(
            out_d[:, 970:1024].rearrange("b (o c) -> o b c", o=1),
            aslice_t[127:128, :].rearrange("p (b c) -> p b c", b=B)[:, :, 0:54])

    nc.compile()
    return nc


def _run(inputs_np, steps=STEPS):
    x = np.asarray(inputs_np["input_data"], np.float32)
    w = np.asarray(inputs_np["weights"], np.float32)
    bias = np.asarray(inputs_np["biases"], np.float32)
    f = np.asarray(inputs_np["from_idx"], np.int32)
    t_ = np.asarray(inputs_np["to_idx"], np.int32)
    in_maps, NT, dts, offs = _preprocess(x, w, bias, f, t_)
    nc = _build(NT, dts, offs, steps, scan_bypass=SCAN_BYPASS, mm8=MM8, refresh3d=REFRESH3D)
    res = bass_utils.run_bass_kernel_spmd(nc, in_maps, list(range(NCD)))
    return np.asarray(res.results[NCD - 1]["out"]).astype(np.float32)


def kernel(**inputs):
    return _run(inputs)


# revision 15
# speedup vs baseline: 1.0755x; 1.0755x over previous
"""Trainium2 Bass kernel for GNN message passing (nn_Brain) — v3.

Reference semantics (per batch b, 20 steps):
    act = zeros(100000); act[:1024] = x_b
    repeat 20: act += tanh(segment_sum(act[from_idx]*w, to_idx) + bias);
               act[:1024] = x_b
    out_b = act[-1024:]

Mapping onto 8 NeuronCores (dest-sharded, batch across partitions):
  * NC r owns dests [r*12500, (r+1)*12500); Q7 core k gathers from source
    chunk k; SBUF partition 16k+b holds chunk k's act for batch b.
  * Per (core, tile): ap_gather acts; mul by int16-held weights; in-place
    cumsum; ap_gather at per-dest segment ends; adjacent diff -> per-core
    partials; [128,8] PE matmul sums the 8 cores per batch; DMA to a DRAM
    total buffer; epilogue adds bias, applies the input clamp via cmask,
    tanh, accumulates, and an AllGather + table-refresh DMAs publish the
    new act slices.

Perf structure (wall ~= 95ms dispatch + ~790ms input upload over the
axon tunnel at ~19ms/MB + ~185ms device exec; measured per-op costs:
ap_gather ~30-45ns/idx on GPSIMD which is the bottleneck engine,
AllGather ~1.1ms, DVE ops ~4-25us):
  * inputs are entropy-packed: 14-bit source indices and 14-bit
    quantized weights, both 8-values-in-7-int16-lanes (weight unpack on
    device, interleaved into step 0 where it hides under the gathers;
    int14 weights give rel err ~8.8e-3 vs the 2e-2 gate — int8 fails at
    0.56 because the 20-step dynamics amplify weight error ~70x).
  * software-pipelined emission: gather(t+1) queues on GPSIMD before
    extract(t), so DVE work (mul+scan) hides under the next gather and
    GPSIMD runs back-to-back; gp/ep/wp pools double-buffered.
  * ones-free in-place cumsum: tensor_tensor_scan(g, g, g, 0, add,
    bypass) (bypass keeps arg0) — saves a [P,T] ones tile.
  * PSUM->SBUF moves on DVE (ScalarE sync hop measured ~40us vs ~5us).
  * epilogue: tot += bias; tot *= cmask; tanh in place; aslice += tot
    (clamped dests get tanh(0)=0 forever; aslice starts at the clamp x).
  * the final step skips AllGather + refresh (nothing consumes them).
"""

import jax
jax.config.update("jax_compilation_cache_dir", "/tmp/jaxcache")
jax.config.update("jax_persistent_cache_min_compile_time_secs", 0)
jax.config.update("jax_persistent_cache_min_entry_size_bytes", 0)

import numpy as np
from contextlib import ExitStack

import concourse.bacc as bacc
import concourse.mybir as mybir
from concourse.tile import TileContext
from concourse import bass_utils
import bass_rust as _bass_rust

def _dep(a, b, reason):
    _bass_rust.add_dep_helper(a.ins, b.ins, True, reason)

F32 = mybir.dt.float32
BF16 = mybir.dt.bfloat16
I16 = mybir.dt.int16

STEPS = 20
IN_SIZE = 1024
OUT_SIZE = 1024
N = 100000
B = 8
NCD = 8
NK = 8
CH = N // NCD
T = 8448
DPX = 704
DMAX = DPX - 1
SLICE_PAD = 12544
PB = SLICE_PAD // 128  # 98
P = 128
STRIP = 16
MM8 = True
SCAN_BYPASS = True
REFRESH3D = False


def _wrap_stream(a):
    NKd, NT, L = a.shape
    aw = a.reshape(NKd, NT, L // 16, 16).transpose(0, 3, 1, 2)
    return np.ascontiguousarray(aw.reshape(NKd * 16, NT * (L // 16)))


def _preprocess(x, w, bias, from_idx, to_idx):
    E = from_idx.shape[0]
    r_arr = (to_idx // CH).astype(np.int32)
    k_arr = (from_idx // CH).astype(np.int32)
    ld = (to_idx % CH).astype(np.int32)
    ls = (from_idx % CH).astype(np.int16)
    strm = r_arr * NK + k_arr
    key = strm.astype(np.int64) * CH + ld
    cnt = np.bincount(key, minlength=64 * CH).reshape(64, CH)
    ccnt = cnt.cumsum(axis=1)

    bounds = []
    s = 0
    base = np.zeros(64, np.int64)
    while s < CH:
        hi = min(s + DMAX, CH)
        if (ccnt[:, hi - 1] - base).max() <= T - 1:
            e = hi
        else:
            lo = s + 1
            h2 = hi
            while lo < h2:
                mid = (lo + h2 + 1) // 2
                if (ccnt[:, mid - 1] - base).max() <= T - 1:
                    lo = mid
                else:
                    h2 = mid - 1
            e = lo
        assert e > s
        bounds.append((s, e))
        base = ccnt[:, e - 1].astype(np.int64).copy()
        s = e
    NT = len(bounds)
    ends = np.array([b[1] for b in bounds])

    tile_of = np.searchsorted(ends, ld, side="right").astype(np.int32)
    # innermost key ls: edges sorted by src within each dest group
    # (order-invariant for the sum; measured ~7% faster ap_gather)
    order = np.lexsort((ls, ld, tile_of, strm))
    so_strm = strm[order]
    so_tile = tile_of[order]
    gkey = so_strm.astype(np.int64) * NT + so_tile
    newg = np.empty(E, bool)
    newg[0] = True
    newg[1:] = gkey[1:] != gkey[:-1]
    gstart = np.flatnonzero(newg)
    gid = np.cumsum(newg) - 1
    pos = np.arange(E, dtype=np.int64) - gstart[gid] + 1
    assert pos.max() <= T - 1

    idx_stream = np.zeros((64, NT, T), np.int16)
    w_stream = np.zeros((64, NT, T), np.int16)
    idx_stream[so_strm, so_tile, pos] = ls[order]
    # 14-bit weight quantization (rel err ~8.8e-3 after 20 chaotic steps;
    # int13 would land at 1.9e-2, over the 2e-2 gate)
    wscale = float(np.abs(w).max()) / 8191.0
    w_stream[so_strm, so_tile, pos] = np.round(w[order] / wscale).astype(np.int16)

    eidx = np.zeros((64, NT, DPX), np.int16)
    for tix, (s0, e0) in enumerate(bounds):
        base_t = ccnt[:, s0 - 1] if s0 > 0 else np.zeros(64, np.int64)
        vals = ccnt[:, s0:e0] - np.asarray(base_t)[:, None]
        eidx[:, tix, 1:1 + (e0 - s0)] = vals.astype(np.int16)

    # Stationary matrix [P, B]: sums the 8 per-core partials of batch b
    # into PSUM partition b; entries are wscale (undo int16 weight quant).
    mmat = np.zeros((P, B if MM8 else P), np.float32)
    for p in range(P):
        if p % 16 < 8:
            mmat[p, p % 16] = wscale

    in_maps = []
    for r in range(NCD):
        sl = slice(r * NK, (r + 1) * NK)
        idx_w = _wrap_stream(idx_stream[sl])
        iw = idx_w.astype(np.uint16).reshape(P, NT, T // 16 // 8, 8)
        lv = np.zeros((P, NT, T // 16 // 8, 7), np.uint16)
        lv[..., 0] = iw[..., 0] | (iw[..., 1] << 14)
        lv[..., 1] = (iw[..., 1] >> 2) | (iw[..., 2] << 12)
        lv[..., 2] = (iw[..., 2] >> 4) | (iw[..., 3] << 10)
        lv[..., 3] = (iw[..., 3] >> 6) | (iw[..., 4] << 8)
        lv[..., 4] = (iw[..., 4] >> 8) | (iw[..., 5] << 6)
        lv[..., 5] = (iw[..., 5] >> 10) | (iw[..., 6] << 4)
        lv[..., 6] = (iw[..., 6] >> 12) | (iw[..., 7] << 2)
        idx_pk = np.ascontiguousarray(
            lv.reshape(P, NT * (T // 16 // 8) * 7)).view(np.int16)
        eidx_w = _wrap_stream(eidx[sl])
        # weights: pack 8 consecutive 14-bit fields into 7 int16 lanes,
        # row-major per core stream (unpacked on device in the prologue)
        wf = (w_stream[sl].reshape(NK, NT * T).astype(np.int64)
              & 0x3FFF).astype(np.uint16).reshape(NK, NT * T // 8, 8)
        wl = np.zeros((NK, NT * T // 8, 7), np.uint16)
        wl[..., 0] = wf[..., 0] | (wf[..., 1] << 14)
        wl[..., 1] = (wf[..., 1] >> 2) | (wf[..., 2] << 12)
        wl[..., 2] = (wf[..., 2] >> 4) | (wf[..., 3] << 10)
        wl[..., 3] = (wf[..., 3] >> 6) | (wf[..., 4] << 8)
        wl[..., 4] = (wf[..., 4] >> 8) | (wf[..., 5] << 6)
        wl[..., 5] = (wf[..., 5] >> 10) | (wf[..., 6] << 4)
        wl[..., 6] = (wf[..., 6] >> 12) | (wf[..., 7] << 2)
        w_hbm = np.ascontiguousarray(
            wl.reshape(NK, NT * T // 8 * 7)).view(np.int16)

        bias_t = np.zeros((P, PB), np.float32)
        for part in range(P):
            l0 = part * PB
            lend = min(l0 + PB, CH)
            if lend > l0:
                bias_t[part, 0:lend - l0] = bias[r * CH + l0:r * CH + lend]

        cmask = np.ones((STRIP, B * PB), np.float32)
        cx = np.zeros((STRIP, B * PB), np.float32)
        if r == 0:
            for part in range(STRIP):
                l0 = part * PB
                ncl = min(IN_SIZE - l0, PB)
                if ncl <= 0:
                    continue
                for b in range(B):
                    cmask[part, b * PB:b * PB + ncl] = 0.0
                    cx[part, b * PB:b * PB + ncl] = x[b, l0:l0 + ncl]
        in_maps.append(dict(
            idxs=idx_pk, eidxs=eidx_w, whbm=w_hbm, xin=x.astype(np.float32),
            biast=bias_t, cmask=cmask, cx=cx, mmat=mmat,
        ))
    dts = [(b[1] - b[0]) for b in bounds]
    offs = [b[0] for b in bounds]
    return in_maps, NT, dts, offs


def _build(NT, dts, offs, steps, scan_bypass=True, mm8=True, refresh3d=False, ab=frozenset()):
    nc = bacc.Bacc("TRN2", target_bir_lowering=False, debug=False,
                   num_devices=NCD)

    PKL = (T // 16 // 8) * 7
    idx_d = nc.dram_tensor("idxs", [P, NT * PKL], I16, kind="ExternalInput")
    eidx_d = nc.dram_tensor("eidxs", [P, NT * (DPX // 16)], I16, kind="ExternalInput")
    PKW = (T // 8) * 7
    w_d = nc.dram_tensor("whbm", [NK, NT * PKW], I16, kind="ExternalInput")
    w_s = nc.dram_tensor("w_scratch", [NK, NT * T], I16)
    x_d = nc.dram_tensor("xin", [B, IN_SIZE], F32, kind="ExternalInput")
    bias_d = nc.dram_tensor("biast", [P, PB], F32, kind="ExternalInput")
    cmask_d = nc.dram_tensor("cmask", [STRIP, B * PB], F32, kind="ExternalInput")
    cx_d = nc.dram_tensor("cx", [STRIP, B * PB], F32, kind="ExternalInput")
    mmat_d = nc.dram_tensor("mmat", [P, B if mm8 else P], F32, kind="ExternalInput")

    total_d = nc.dram_tensor("total_dram", [B, SLICE_PAD], F32)
    ag_in = nc.dram_tensor("ag_in", [B, SLICE_PAD], F32)
    ag_out = nc.dram_tensor("ag_out", [NCD * B, SLICE_PAD], F32,
                            addr_space="Shared")
    out_d = nc.dram_tensor("out", [B, OUT_SIZE], F32, kind="ExternalOutput")

    ADD, BYP = mybir.AluOpType.add, mybir.AluOpType.bypass

    with TileContext(nc) as tc, ExitStack() as ctx:
        cpool = ctx.enter_context(tc.tile_pool(name="const", bufs=1))
        wp = ctx.enter_context(tc.tile_pool(name="wp", bufs=2))
        gp = ctx.enter_context(tc.tile_pool(name="gp", bufs=2))
        ep = ctx.enter_context(tc.tile_pool(name="ep", bufs=2))
        dp = ctx.enter_context(tc.tile_pool(name="dp", bufs=2))
        pp = ctx.enter_context(tc.tile_pool(name="pp", bufs=2, space="PSUM"))
        pkp = ctx.enter_context(tc.tile_pool(name="pkp", bufs=2))
        tmpp = ctx.enter_context(tc.tile_pool(name="tmpp", bufs=2))
        sp = ctx.enter_context(tc.tile_pool(name="sp", bufs=2))
        slp = ctx.enter_context(tc.tile_pool(name="slp", bufs=1))

        # Resident data
        table_t = cpool.tile([P, CH], F32)
        nc.vector.memset(table_t[:], 0.0)
        nc.sync.dma_start(table_t[0:B, 0:IN_SIZE], x_d[:])
        mmat_t = cpool.tile([P, B if mm8 else P], F32)
        nc.sync.dma_start(mmat_t[:], mmat_d[:])
        eidx_t = cpool.tile([P, NT * (DPX // 16)], I16)
        nc.sync.dma_start(eidx_t[:], eidx_d[:])
        idx_t = cpool.tile([P, NT * (T // 16)], I16)
        zb_t = cpool.tile([P, 1], F32)   # dummy data1 for the bypass scan
        nc.vector.memset(zb_t[:], 0.0)
        zb_bc = zb_t[:].broadcast_to((P, T))
        ones_t = None
        if not scan_bypass:
            ones_t = cpool.tile([P, T], BF16)
            nc.vector.memset(ones_t[:], 1.0)
        NG = T // 16 // 8
        AND, SHR, SHL, OR = (mybir.AluOpType.bitwise_and,
                             mybir.AluOpType.logical_shift_right,
                             mybir.AluOpType.logical_shift_left,
                             mybir.AluOpType.bitwise_or)
        for t in range(NT):
            pk_t = pkp.tile([P, PKL], I16, tag="pk")
            nc.sync.dma_start(pk_t[:], idx_d[:, t * PKL:(t + 1) * PKL])
            pkv = pk_t[:].rearrange("p (g l) -> p g l", l=7)
            ov = idx_t[:, t * (T // 16):(t + 1) * (T // 16)].rearrange(
                "p (g e) -> p g e", e=8)
            nc.vector.tensor_single_scalar(
                ov[:, :, 0:1], pkv[:, :, 0:1], 0x3FFF, AND)
            nc.vector.tensor_scalar(
                ov[:, :, 7:8], pkv[:, :, 6:7], 2, 0x3FFF, SHR, AND)
            for o in range(1, 7):
                tmp_t = tmpp.tile([P, NG], I16, tag="tmp")
                tm2_t = tmpp.tile([P, NG], I16, tag="tm2")
                tv = tmp_t[:].rearrange("p (g o) -> p g o", o=1)
                tv2 = tm2_t[:].rearrange("p (g o) -> p g o", o=1)
                nc.vector.tensor_scalar(
                    tv, pkv[:, :, o - 1:o], 16 - 2 * o, (1 << (2 * o)) - 1,
                    SHR, AND)
                nc.vector.tensor_single_scalar(
                    tv2, pkv[:, :, o:o + 1], 2 * o, SHL)
                nc.vector.tensor_tensor(tv, tv, tv2, OR)
                nc.vector.tensor_single_scalar(
                    ov[:, :, o:o + 1], tv, 0x3FFF, AND)

        # 14-bit weight unpack (emitted per tile inside step 0, where the
        # ~200us of DVE work per tile hides under the ~293us GPSIMD gather).
        # Reuses wp-pool tiles plus one small tmp tag; sign extension via
        # mask/xor/sub, correct for a 32-bit sign-extending ALU.
        XOR, SUB = mybir.AluOpType.bitwise_xor, mybir.AluOpType.subtract
        GW = T // 8

        def emit_w_unpack(t):
            wq_t = wp.tile([P, T], I16, tag="w")
            nc.sync.dma_start(wq_t[0:NK, 0:PKW], w_d[:, t * PKW:(t + 1) * PKW])
            wu_t = wp.tile([P, T], I16, tag="w")
            qv = wq_t[0:NK, 0:PKW].rearrange("p (g l) -> p g l", l=7)
            uv = wu_t[0:NK, :].rearrange("p (g e) -> p g e", e=8)
            nc.vector.tensor_scalar(
                uv[:, :, 0:1], qv[:, :, 0:1], 0x3FFF, 0x2000, AND, XOR)
            nc.vector.tensor_single_scalar(
                uv[:, :, 0:1], uv[:, :, 0:1], 0x2000, SUB)
            nc.vector.tensor_scalar(
                uv[:, :, 7:8], qv[:, :, 6:7], 2, 0x3FFF, SHR, AND)
            nc.vector.tensor_single_scalar(
                uv[:, :, 7:8], uv[:, :, 7:8], 0x2000, XOR)
            nc.vector.tensor_single_scalar(
                uv[:, :, 7:8], uv[:, :, 7:8], 0x2000, SUB)
            for o in range(1, 7):
                wt_t = tmpp.tile([P, GW], I16, tag="wt")
                wv = wt_t[0:NK, :].rearrange("p (g o) -> p g o", o=1)
                nc.vector.tensor_scalar(
                    wv, qv[:, :, o - 1:o], 16 - 2 * o, (1 << (2 * o)) - 1,
                    SHR, AND)
                nc.vector.tensor_single_scalar(
                    uv[:, :, o:o + 1], qv[:, :, o:o + 1], 2 * o, SHL)
                nc.vector.tensor_tensor(uv[:, :, o:o + 1], uv[:, :, o:o + 1],
                                        wv, OR)
                nc.vector.tensor_scalar(
                    uv[:, :, o:o + 1], uv[:, :, o:o + 1], 0x3FFF, 0x2000,
                    AND, XOR)
                nc.vector.tensor_single_scalar(
                    uv[:, :, o:o + 1], uv[:, :, o:o + 1], 0x2000, SUB)
            return nc.sync.dma_start(w_s[:, t * T:(t + 1) * T], wu_t[0:NK, :])

        cmask_t = slp.tile([P, B * PB], F32)
        nc.vector.memset(cmask_t[:], 1.0)
        nc.sync.dma_start(cmask_t[0:STRIP, :], cmask_d[:])
        aslice_t = slp.tile([P, B * PB], F32)
        nc.vector.memset(aslice_t[:], 0.0)
        nc.sync.dma_start(aslice_t[0:STRIP, :], cx_d[:])
        bias_s = slp.tile([P, PB], F32)
        nc.sync.dma_start(bias_s[:], bias_d[:])
        bias_f = slp.tile([P, B * PB], F32)
        for b in range(B):
            nc.vector.tensor_copy(bias_f[:, b * PB:(b + 1) * PB], bias_s[:])

        prev_state = {"readback": None, "collective": None}

        def step_body(step, tiles=None):
            if tiles is None:
                tiles = list(range(NT))
            NL = len(tiles)
            out_dmas = []
            cur = {}
            order2 = "serialext" in ab
            for i in range(NL + 1):
                t = tiles[i] if i < NL else -1
                if order2 and i == NL:
                    break
                if i < NL:
                    if step == 0:
                        wu_dma = emit_w_unpack(t)
                    w_t = wp.tile([P, T], I16, tag="w")
                    if "smallwdma" in ab:
                        w_src = w_s[:, t * T:t * T + 64].rearrange(
                            "k (o t) -> k o t", o=1).broadcast_to((NK, 16, 64))
                        wdma = nc.sync.dma_start(w_t[:, 0:64], w_src)
                    else:
                        w_src = w_s[:, t * T:(t + 1) * T].rearrange(
                            "k (o t) -> k o t", o=1).broadcast_to((NK, 16, T))
                        wdma = nc.sync.dma_start(w_t[:], w_src)
                    if step == 0:
                        _dep(wdma, wu_dma, "RAW w_scratch")
                    g_t = gp.tile([P, T], F32, tag="g")
                    if "nogather" in ab:
                        nc.vector.memset(g_t[:], 0.001)
                    else:
                        nc.gpsimd.ap_gather(
                            g_t[:], table_t[:],
                            idx_t[:, t * (T // 16):(t + 1) * (T // 16)],
                            channels=P, num_elems=CH, d=1, num_idxs=T)
                if (i >= 1 and not order2) or (order2 and False):
                    tp = tiles[i - 1]
                    extr_t = ep.tile([P, DPX], F32, tag="extr")
                    if "noextract" in ab:
                        nc.vector.memset(extr_t[:], 0.001)
                    else:
                        nc.gpsimd.ap_gather(
                            extr_t[:], cur["g"],
                            eidx_t[:, tp * (DPX // 16):(tp + 1) * (DPX // 16)],
                            channels=P, num_elems=T, d=1, num_idxs=DPX)
                if i < NL:
                    if "nomul" not in ab:
                        nc.vector.tensor_mul(g_t[:], g_t[:], w_t[:])
                    if "noscan" in ab:
                        pass
                    elif scan_bypass:
                        nc.vector.tensor_tensor_scan(
                            g_t[:], g_t[:], zb_bc, 0.0, ADD, BYP)
                    else:
                        nc.vector.tensor_tensor_scan(
                            g_t[:], ones_t[:], g_t[:], 0.0,
                            mybir.AluOpType.mult, mybir.AluOpType.add)
                if order2 and i < NL:
                    tp = t
                    extr_t = ep.tile([P, DPX], F32, tag="extr")
                    if "noextract" in ab:
                        nc.vector.memset(extr_t[:], 0.001)
                    else:
                        nc.gpsimd.ap_gather(
                            extr_t[:], g_t[:],
                            eidx_t[:, tp * (DPX // 16):(tp + 1) * (DPX // 16)],
                            channels=P, num_elems=T, d=1, num_idxs=DPX)
                if (i >= 1 and not order2) or (order2 and i < NL):
                    if not order2:
                        tp = tiles[i - 1]
                    diff_t = dp.tile([P, DPX - 1], F32, tag="diff")
                    nc.vector.tensor_sub(diff_t[:], extr_t[:, 1:DPX],
                                         extr_t[:, 0:DPX - 1])
                    ps_t = pp.tile([B if mm8 else P, DPX - 1], F32, tag="ps")
                    nc.tensor.matmul(ps_t[:, 0:512], mmat_t[:],
                                     diff_t[:, 0:512], start=True, stop=True)
                    if dts[tp] > 512:
                        nc.tensor.matmul(ps_t[:, 512:DPX - 1], mmat_t[:],
                                         diff_t[:, 512:DPX - 1],
                                         start=True, stop=True)
                    st_t = sp.tile([B, DPX - 1], F32, tag="st")
                    nc.vector.tensor_copy(st_t[:, 0:dts[tp]],
                                          ps_t[0:B, 0:dts[tp]])
                    od = nc.sync.dma_start(
                        total_d[:, offs[tp]:offs[tp] + dts[tp]],
                        st_t[:, 0:dts[tp]])
                    out_dmas.append(od)
                    if prev_state["readback"] is not None:
                        _dep(od, prev_state["readback"],
                             "WAR total_d across steps")
                if i < NL:
                    cur = {"g": g_t[:]}

            # Epilogue
            if "noepi" in ab:
                prev_state["readback"] = None
                return
            tot_t = slp.tile([P, B * PB], F32, tag="tot")
            rb = nc.sync.dma_start(
                tot_t[:].rearrange("p (b c) -> p b c", b=B),
                total_d[:].rearrange("b (p c) -> p b c", p=P))
            for od in out_dmas:
                _dep(rb, od, "RAW total_d")
            nc.vector.tensor_add(tot_t[:], tot_t[:], bias_f[:])
            nc.vector.tensor_mul(tot_t[:], tot_t[:], cmask_t[:])
            nc.scalar.activation(tot_t[:], tot_t[:],
                                 mybir.ActivationFunctionType.Tanh)
            nc.vector.tensor_add(aslice_t[:], aslice_t[:], tot_t[:])
            prev_state["readback"] = rb
            if step == steps - 1:
                return  # nothing consumes the collective after the last step
            if "nocc" in ab:
                return
            wb = nc.sync.dma_start(
                ag_in[:].rearrange("b (p c) -> p b c", p=P),
                aslice_t[:].rearrange("p (b c) -> p b c", b=B))
            if prev_state["collective"] is not None:
                _dep(wb, prev_state["collective"], "WAR ag_in")
            cc = nc.gpsimd.collective_compute(
                "AllGather", mybir.AluOpType.bypass,
                replica_groups=[list(range(NCD))],
                ins=[ag_in[:]], outs=[ag_out[:]])
            _dep(cc, wb, "RAW ag_in")
            if refresh3d:
                tr = nc.sync.dma_start(
                    table_t[:].rearrange("(k s) c -> k s c", s=16)[:, 0:B, :],
                    ag_out[:].rearrange("(k b) c -> k b c", k=NK)[:, :, 0:CH])
                _dep(tr, cc, "RAW ag_out")
            else:
                for k in range(NK):
                    tr = nc.sync.dma_start(
                        table_t[16 * k:16 * k + B, :],
                        ag_out[B * k:B * (k + 1), 0:CH])
                    _dep(tr, cc, "RAW ag_out")
            prev_state["collective"] = cc

        # Tiles needed by the final output (dests [tail0_lo, CH), where
        # tail0_lo = 117*PB rounds the 1024-dest output tail down to the
        # partition holding its first element).
        tail_lo = (CH - OUT_SIZE) // PB * PB
        tail_tiles = [t for t in range(NT) if offs[t] + dts[t] > tail_lo]
        for s in range(steps):
            if s == steps - 1 and s > 0:
                step_body(s, tiles=tail_tiles)
            else:
                step_body(s)

        # Final output from aslice_t (tail0 = 11476 = 117*98 + 10).
        nc.sync.dma_start(
            out_d[:, 0:88].rearrange("b (o c) -> o b c", o=1),
            aslice_t[117:118, :].rearrange("p (b c) -> p b c", b=B)[:, :, 10:98])
        nc.sync.dma_start(
            out_d[:, 88:970].rearrange("b (p c) -> p b c", p=9),
            aslice_t[118:127, :].rearrange("p (b c) -> p b c", b=B))
        nc.sync.dma_start(
            out_d[:, 970:1024].rearrange("b (o c) -> o b c", o=1),
            aslice_t[127:128, :].rearrange("p (b c) -> p b c", b=B)[:, :, 0:54])

    nc.compile()
    return nc


def _run(inputs_np, steps=STEPS):
    x = np.asarray(inputs_np["input_data"], np.float32)
    w = np.asarray(inputs_np["weights"], np.float32)
    bias = np.asarray(inputs_np["biases"], np.float32)
    f = np.asarray(inputs_np["from_idx"], np.int32)
    t_ = np.asarray(inputs_np["to_idx"], np.int32)
    in_maps, NT, dts, offs = _preprocess(x, w, bias, f, t_)
    nc = _build(NT, dts, offs, steps, scan_bypass=SCAN_BYPASS, mm8=MM8, refresh3d=REFRESH3D)
    res = bass_utils.run_bass_kernel_spmd(nc, in_maps, list(range(NCD)))
    return np.asarray(res.results[NCD - 1]["out"]).astype(np.float32)


def kernel(**inputs):
    return _run(inputs)


# revision 16
# speedup vs baseline: 1.1007x; 1.0234x over previous
"""Trainium2 Bass kernel for GNN message passing (nn_Brain) — v3.

Reference semantics (per batch b, 20 steps):
    act = zeros(100000); act[:1024] = x_b
    repeat 20: act += tanh(segment_sum(act[from_idx]*w, to_idx) + bias);
               act[:1024] = x_b
    out_b = act[-1024:]

Mapping onto 8 NeuronCores (dest-sharded, batch across partitions):
  * NC r owns dests [r*12500, (r+1)*12500); Q7 core k gathers from source
    chunk k; SBUF partition 16k+b holds chunk k's act for batch b.
  * Per (core, tile): ap_gather acts; mul by int16-held weights; in-place
    cumsum; ap_gather at per-dest segment ends; adjacent diff -> per-core
    partials; [128,8] PE matmul sums the 8 cores per batch; DMA to a DRAM
    total buffer; epilogue adds bias, applies the input clamp via cmask,
    tanh, accumulates, and an AllGather + table-refresh DMAs publish the
    new act slices.

Perf structure (wall ~= 95ms dispatch + ~790ms input upload over the
axon tunnel at ~19ms/MB + ~185ms device exec; measured per-op costs:
ap_gather ~30-45ns/idx on GPSIMD which is the bottleneck engine,
AllGather ~1.1ms, DVE ops ~4-25us):
  * inputs are entropy-packed: 14-bit source indices and 14-bit
    quantized weights, both 8-values-in-7-int16-lanes (weight unpack on
    device, interleaved into step 0 where it hides under the gathers;
    int14 weights give rel err ~8.8e-3 vs the 2e-2 gate — int8 fails at
    0.56 because the 20-step dynamics amplify weight error ~70x).
  * software-pipelined emission: gather(t+1) queues on GPSIMD before
    extract(t), so DVE work (mul+scan) hides under the next gather and
    GPSIMD runs back-to-back; gp/ep/wp pools double-buffered.
  * ones-free in-place cumsum: tensor_tensor_scan(g, g, g, 0, add,
    bypass) (bypass keeps arg0) — saves a [P,T] ones tile.
  * PSUM->SBUF moves on DVE (ScalarE sync hop measured ~40us vs ~5us).
  * epilogue: tot += bias; tot *= cmask; tanh in place; aslice += tot
    (clamped dests get tanh(0)=0 forever; aslice starts at the clamp x).
  * the final step skips AllGather + refresh (nothing consumes them).
"""

import jax
jax.config.update("jax_compilation_cache_dir", "/tmp/jaxcache")
jax.config.update("jax_persistent_cache_min_compile_time_secs", 0)
jax.config.update("jax_persistent_cache_min_entry_size_bytes", 0)

import numpy as np
from contextlib import ExitStack

import concourse.bacc as bacc
import concourse.mybir as mybir
from concourse.tile import TileContext
from concourse import bass_utils
import bass_rust as _bass_rust

def _dep(a, b, reason):
    _bass_rust.add_dep_helper(a.ins, b.ins, True, reason)

F32 = mybir.dt.float32
BF16 = mybir.dt.bfloat16
I16 = mybir.dt.int16

STEPS = 20
IN_SIZE = 1024
OUT_SIZE = 1024
N = 100000
B = 8
NCD = 8
NK = 8
CH = N // NCD
T = 8448
DPX = 704
DMAX = DPX - 1
SLICE_PAD = 12544
PB = SLICE_PAD // 128  # 98
P = 128
STRIP = 16
MM8 = True
SCAN_BYPASS = True
REFRESH3D = False


def _wrap_stream(a):
    NKd, NT, L = a.shape
    aw = a.reshape(NKd, NT, L // 16, 16).transpose(0, 3, 1, 2)
    return np.ascontiguousarray(aw.reshape(NKd * 16, NT * (L // 16)))


def _preprocess(x, w, bias, from_idx, to_idx):
    E = from_idx.shape[0]
    r_arr = (to_idx // CH).astype(np.int32)
    k_arr = (from_idx // CH).astype(np.int32)
    ld = (to_idx % CH).astype(np.int32)
    ls = (from_idx % CH).astype(np.int16)
    strm = r_arr * NK + k_arr
    key = strm.astype(np.int64) * CH + ld
    cnt = np.bincount(key, minlength=64 * CH).reshape(64, CH)
    ccnt = cnt.cumsum(axis=1)

    bounds = []
    s = 0
    base = np.zeros(64, np.int64)
    while s < CH:
        hi = min(s + DMAX, CH)
        if (ccnt[:, hi - 1] - base).max() <= T - 1:
            e = hi
        else:
            lo = s + 1
            h2 = hi
            while lo < h2:
                mid = (lo + h2 + 1) // 2
                if (ccnt[:, mid - 1] - base).max() <= T - 1:
                    lo = mid
                else:
                    h2 = mid - 1
            e = lo
        assert e > s
        bounds.append((s, e))
        base = ccnt[:, e - 1].astype(np.int64).copy()
        s = e
    NT = len(bounds)
    ends = np.array([b[1] for b in bounds])

    tile_of = np.searchsorted(ends, ld, side="right").astype(np.int32)
    # innermost key ls: edges sorted by src within each dest group
    # (order-invariant for the sum; measured ~7% faster ap_gather)
    order = np.lexsort((ls, ld, tile_of, strm))
    so_strm = strm[order]
    so_tile = tile_of[order]
    gkey = so_strm.astype(np.int64) * NT + so_tile
    newg = np.empty(E, bool)
    newg[0] = True
    newg[1:] = gkey[1:] != gkey[:-1]
    gstart = np.flatnonzero(newg)
    gid = np.cumsum(newg) - 1
    pos = np.arange(E, dtype=np.int64) - gstart[gid] + 1
    assert pos.max() <= T - 1

    idx_stream = np.zeros((64, NT, T), np.int16)
    w_stream = np.zeros((64, NT, T), np.int16)
    idx_stream[so_strm, so_tile, pos] = ls[order]
    # 14-bit weight quantization (rel err ~8.8e-3 after 20 chaotic steps;
    # int13 would land at 1.9e-2, over the 2e-2 gate).  Edges into clamped
    # dests (< IN_SIZE) get w=0 and their bias entries are zeroed below, so
    # tanh(total)=0 there and the clamp needs no per-step mask multiply.
    wscale = float(np.abs(w).max()) / 8191.0
    w_eff = np.where(to_idx < IN_SIZE, 0.0, w)
    w_stream[so_strm, so_tile, pos] = np.round(
        w_eff[order] / wscale).astype(np.int16)

    eidx = np.zeros((64, NT, DPX), np.int16)
    for tix, (s0, e0) in enumerate(bounds):
        base_t = ccnt[:, s0 - 1] if s0 > 0 else np.zeros(64, np.int64)
        vals = ccnt[:, s0:e0] - np.asarray(base_t)[:, None]
        eidx[:, tix, 1:1 + (e0 - s0)] = vals.astype(np.int16)

    # Stationary matrix [P, B]: sums the 8 per-core partials of batch b
    # into PSUM partition b; entries are wscale (undo int16 weight quant).
    mmat = np.zeros((P, B if MM8 else P), np.float32)
    for p in range(P):
        if p % 16 < 8:
            mmat[p, p % 16] = wscale

    in_maps = []
    for r in range(NCD):
        sl = slice(r * NK, (r + 1) * NK)
        idx_w = _wrap_stream(idx_stream[sl])
        iw = idx_w.astype(np.uint16).reshape(P, NT, T // 16 // 8, 8)
        lv = np.zeros((P, NT, T // 16 // 8, 7), np.uint16)
        lv[..., 0] = iw[..., 0] | (iw[..., 1] << 14)
        lv[..., 1] = (iw[..., 1] >> 2) | (iw[..., 2] << 12)
        lv[..., 2] = (iw[..., 2] >> 4) | (iw[..., 3] << 10)
        lv[..., 3] = (iw[..., 3] >> 6) | (iw[..., 4] << 8)
        lv[..., 4] = (iw[..., 4] >> 8) | (iw[..., 5] << 6)
        lv[..., 5] = (iw[..., 5] >> 10) | (iw[..., 6] << 4)
        lv[..., 6] = (iw[..., 6] >> 12) | (iw[..., 7] << 2)
        idx_pk = np.ascontiguousarray(
            lv.reshape(P, NT * (T // 16 // 8) * 7)).view(np.int16)
        eidx_w = _wrap_stream(eidx[sl])
        # weights: pack 8 consecutive 14-bit fields into 7 int16 lanes,
        # row-major per core stream (unpacked on device in the prologue)
        wf = (w_stream[sl].reshape(NK, NT * T).astype(np.int64)
              & 0x3FFF).astype(np.uint16).reshape(NK, NT * T // 8, 8)
        wl = np.zeros((NK, NT * T // 8, 7), np.uint16)
        wl[..., 0] = wf[..., 0] | (wf[..., 1] << 14)
        wl[..., 1] = (wf[..., 1] >> 2) | (wf[..., 2] << 12)
        wl[..., 2] = (wf[..., 2] >> 4) | (wf[..., 3] << 10)
        wl[..., 3] = (wf[..., 3] >> 6) | (wf[..., 4] << 8)
        wl[..., 4] = (wf[..., 4] >> 8) | (wf[..., 5] << 6)
        wl[..., 5] = (wf[..., 5] >> 10) | (wf[..., 6] << 4)
        wl[..., 6] = (wf[..., 6] >> 12) | (wf[..., 7] << 2)
        w_hbm = np.ascontiguousarray(
            wl.reshape(NK, NT * T // 8 * 7)).view(np.int16)

        bias_eff = bias.copy()
        bias_eff[:IN_SIZE] = 0.0  # clamped dests: tanh input must be 0
        bias_t = np.zeros((P, PB), np.float32)
        for part in range(P):
            l0 = part * PB
            lend = min(l0 + PB, CH)
            if lend > l0:
                bias_t[part, 0:lend - l0] = bias_eff[r * CH + l0:r * CH + lend]

        cx = np.zeros((STRIP, B * PB), np.float32)
        if r == 0:
            for part in range(STRIP):
                l0 = part * PB
                ncl = min(IN_SIZE - l0, PB)
                if ncl <= 0:
                    continue
                for b in range(B):
                    cx[part, b * PB:b * PB + ncl] = x[b, l0:l0 + ncl]
        in_maps.append(dict(
            idxs=idx_pk, eidxs=eidx_w, whbm=w_hbm, xin=x.astype(np.float32),
            biast=bias_t, cx=cx, mmat=mmat,
        ))
    dts = [(b[1] - b[0]) for b in bounds]
    offs = [b[0] for b in bounds]
    return in_maps, NT, dts, offs


def _build(NT, dts, offs, steps, scan_bypass=True, mm8=True, refresh3d=False, ab=frozenset()):
    nc = bacc.Bacc("TRN2", target_bir_lowering=False, debug=False,
                   num_devices=NCD)

    PKL = (T // 16 // 8) * 7
    idx_d = nc.dram_tensor("idxs", [P, NT * PKL], I16, kind="ExternalInput")
    eidx_d = nc.dram_tensor("eidxs", [P, NT * (DPX // 16)], I16, kind="ExternalInput")
    PKW = (T // 8) * 7
    w_d = nc.dram_tensor("whbm", [NK, NT * PKW], I16, kind="ExternalInput")
    w_s = nc.dram_tensor("w_scratch", [NK, NT * T], I16)
    x_d = nc.dram_tensor("xin", [B, IN_SIZE], F32, kind="ExternalInput")
    bias_d = nc.dram_tensor("biast", [P, PB], F32, kind="ExternalInput")
    cx_d = nc.dram_tensor("cx", [STRIP, B * PB], F32, kind="ExternalInput")
    mmat_d = nc.dram_tensor("mmat", [P, B if mm8 else P], F32, kind="ExternalInput")

    total_d = nc.dram_tensor("total_dram", [B, SLICE_PAD], F32)
    ag_in = nc.dram_tensor("ag_in", [B, SLICE_PAD], F32)
    ag_out = nc.dram_tensor("ag_out", [NCD * B, SLICE_PAD], F32,
                            addr_space="Shared")
    out_d = nc.dram_tensor("out", [B, OUT_SIZE], F32, kind="ExternalOutput")

    ADD, BYP = mybir.AluOpType.add, mybir.AluOpType.bypass

    with TileContext(nc) as tc, ExitStack() as ctx:
        cpool = ctx.enter_context(tc.tile_pool(name="const", bufs=1))
        wp = ctx.enter_context(tc.tile_pool(name="wp", bufs=2))
        gp = ctx.enter_context(tc.tile_pool(name="gp", bufs=2))
        ep = ctx.enter_context(tc.tile_pool(name="ep", bufs=2))
        dp = ctx.enter_context(tc.tile_pool(name="dp", bufs=2))
        pp = ctx.enter_context(tc.tile_pool(name="pp", bufs=2, space="PSUM"))
        pkp = ctx.enter_context(tc.tile_pool(name="pkp", bufs=2))
        tmpp = ctx.enter_context(tc.tile_pool(name="tmpp", bufs=2))
        sp = ctx.enter_context(tc.tile_pool(name="sp", bufs=2))
        slp = ctx.enter_context(tc.tile_pool(name="slp", bufs=1))

        # Resident data
        table_t = cpool.tile([P, CH], F32)
        nc.vector.memset(table_t[:], 0.0)
        nc.sync.dma_start(table_t[0:B, 0:IN_SIZE], x_d[:])
        mmat_t = cpool.tile([P, B if mm8 else P], F32)
        nc.sync.dma_start(mmat_t[:], mmat_d[:])
        eidx_t = cpool.tile([P, NT * (DPX // 16)], I16)
        nc.sync.dma_start(eidx_t[:], eidx_d[:])
        idx_t = cpool.tile([P, NT * (T // 16)], I16)
        zb_t = cpool.tile([P, 1], F32)   # dummy data1 for the bypass scan
        nc.vector.memset(zb_t[:], 0.0)
        zb_bc = zb_t[:].broadcast_to((P, T))
        ones_t = None
        if not scan_bypass:
            ones_t = cpool.tile([P, T], BF16)
            nc.vector.memset(ones_t[:], 1.0)
        NG = T // 16 // 8
        AND, SHR, SHL, OR = (mybir.AluOpType.bitwise_and,
                             mybir.AluOpType.logical_shift_right,
                             mybir.AluOpType.logical_shift_left,
                             mybir.AluOpType.bitwise_or)
        for t in range(NT):
            pk_t = pkp.tile([P, PKL], I16, tag="pk")
            nc.sync.dma_start(pk_t[:], idx_d[:, t * PKL:(t + 1) * PKL])
            pkv = pk_t[:].rearrange("p (g l) -> p g l", l=7)
            ov = idx_t[:, t * (T // 16):(t + 1) * (T // 16)].rearrange(
                "p (g e) -> p g e", e=8)
            nc.vector.tensor_single_scalar(
                ov[:, :, 0:1], pkv[:, :, 0:1], 0x3FFF, AND)
            nc.vector.tensor_scalar(
                ov[:, :, 7:8], pkv[:, :, 6:7], 2, 0x3FFF, SHR, AND)
            for o in range(1, 7):
                tmp_t = tmpp.tile([P, NG], I16, tag="tmp")
                tm2_t = tmpp.tile([P, NG], I16, tag="tm2")
                tv = tmp_t[:].rearrange("p (g o) -> p g o", o=1)
                tv2 = tm2_t[:].rearrange("p (g o) -> p g o", o=1)
                nc.vector.tensor_scalar(
                    tv, pkv[:, :, o - 1:o], 16 - 2 * o, (1 << (2 * o)) - 1,
                    SHR, AND)
                nc.vector.tensor_single_scalar(
                    tv2, pkv[:, :, o:o + 1], 2 * o, SHL)
                nc.vector.tensor_tensor(tv, tv, tv2, OR)
                nc.vector.tensor_single_scalar(
                    ov[:, :, o:o + 1], tv, 0x3FFF, AND)

        # 14-bit weight unpack (emitted per tile inside step 0, where the
        # ~200us of DVE work per tile hides under the ~293us GPSIMD gather).
        # Reuses wp-pool tiles plus one small tmp tag; sign extension via
        # mask/xor/sub, correct for a 32-bit sign-extending ALU.
        XOR, SUB = mybir.AluOpType.bitwise_xor, mybir.AluOpType.subtract
        GW = T // 8

        def emit_w_unpack(t):
            wq_t = wp.tile([P, T], I16, tag="w")
            nc.sync.dma_start(wq_t[0:NK, 0:PKW], w_d[:, t * PKW:(t + 1) * PKW])
            wu_t = wp.tile([P, T], I16, tag="w")
            qv = wq_t[0:NK, 0:PKW].rearrange("p (g l) -> p g l", l=7)
            uv = wu_t[0:NK, :].rearrange("p (g e) -> p g e", e=8)
            nc.vector.tensor_scalar(
                uv[:, :, 0:1], qv[:, :, 0:1], 0x3FFF, 0x2000, AND, XOR)
            nc.vector.tensor_single_scalar(
                uv[:, :, 0:1], uv[:, :, 0:1], 0x2000, SUB)
            nc.vector.tensor_scalar(
                uv[:, :, 7:8], qv[:, :, 6:7], 2, 0x3FFF, SHR, AND)
            nc.vector.tensor_single_scalar(
                uv[:, :, 7:8], uv[:, :, 7:8], 0x2000, XOR)
            nc.vector.tensor_single_scalar(
                uv[:, :, 7:8], uv[:, :, 7:8], 0x2000, SUB)
            for o in range(1, 7):
                wt_t = tmpp.tile([P, GW], I16, tag="wt")
                wv = wt_t[0:NK, :].rearrange("p (g o) -> p g o", o=1)
                nc.vector.tensor_scalar(
                    wv, qv[:, :, o - 1:o], 16 - 2 * o, (1 << (2 * o)) - 1,
                    SHR, AND)
                nc.vector.tensor_single_scalar(
                    uv[:, :, o:o + 1], qv[:, :, o:o + 1], 2 * o, SHL)
                nc.vector.tensor_tensor(uv[:, :, o:o + 1], uv[:, :, o:o + 1],
                                        wv, OR)
                nc.vector.tensor_scalar(
                    uv[:, :, o:o + 1], uv[:, :, o:o + 1], 0x3FFF, 0x2000,
                    AND, XOR)
                nc.vector.tensor_single_scalar(
                    uv[:, :, o:o + 1], uv[:, :, o:o + 1], 0x2000, SUB)
            return nc.sync.dma_start(w_s[:, t * T:(t + 1) * T], wu_t[0:NK, :])

        aslice_t = slp.tile([P, B * PB], F32)
        nc.vector.memset(aslice_t[:], 0.0)
        nc.sync.dma_start(aslice_t[0:STRIP, :], cx_d[:])
        bias_s = slp.tile([P, PB], F32)
        nc.sync.dma_start(bias_s[:], bias_d[:])
        bias_f = slp.tile([P, B * PB], F32)
        for b in range(B):
            nc.vector.tensor_copy(bias_f[:, b * PB:(b + 1) * PB], bias_s[:])

        prev_state = {"readback": None, "collective": None}

        def step_body(step, tiles=None):
            if tiles is None:
                tiles = list(range(NT))
            NL = len(tiles)
            out_dmas = []
            cur = {}
            order2 = "serialext" in ab
            for i in range(NL + 1):
                t = tiles[i] if i < NL else -1
                if order2 and i == NL:
                    break
                if i < NL:
                    if step == 0:
                        wu_dma = emit_w_unpack(t)
                    w_t = wp.tile([P, T], I16, tag="w")
                    if "smallwdma" in ab:
                        w_src = w_s[:, t * T:t * T + 64].rearrange(
                            "k (o t) -> k o t", o=1).broadcast_to((NK, 16, 64))
                        wdma = nc.sync.dma_start(w_t[:, 0:64], w_src)
                    else:
                        w_src = w_s[:, t * T:(t + 1) * T].rearrange(
                            "k (o t) -> k o t", o=1).broadcast_to((NK, 16, T))
                        wdma = nc.sync.dma_start(w_t[:], w_src)
                    if step == 0:
                        _dep(wdma, wu_dma, "RAW w_scratch")
                    g_t = gp.tile([P, T], F32, tag="g")
                    if "nogather" in ab:
                        nc.vector.memset(g_t[:], 0.001)
                    else:
                        nc.gpsimd.ap_gather(
                            g_t[:], table_t[:],
                            idx_t[:, t * (T // 16):(t + 1) * (T // 16)],
                            channels=P, num_elems=CH, d=1, num_idxs=T)
                if (i >= 1 and not order2) or (order2 and False):
                    tp = tiles[i - 1]
                    extr_t = ep.tile([P, DPX], F32, tag="extr")
                    if "noextract" in ab:
                        nc.vector.memset(extr_t[:], 0.001)
                    else:
                        nc.gpsimd.ap_gather(
                            extr_t[:], cur["g"],
                            eidx_t[:, tp * (DPX // 16):(tp + 1) * (DPX // 16)],
                            channels=P, num_elems=T, d=1, num_idxs=DPX)
                if i < NL:
                    if "nomul" not in ab:
                        nc.vector.tensor_mul(g_t[:], g_t[:], w_t[:])
                    if "noscan" in ab:
                        pass
                    elif scan_bypass:
                        nc.vector.tensor_tensor_scan(
                            g_t[:], g_t[:], zb_bc, 0.0, ADD, BYP)
                    else:
                        nc.vector.tensor_tensor_scan(
                            g_t[:], ones_t[:], g_t[:], 0.0,
                            mybir.AluOpType.mult, mybir.AluOpType.add)
                if order2 and i < NL:
                    tp = t
                    extr_t = ep.tile([P, DPX], F32, tag="extr")
                    if "noextract" in ab:
                        nc.vector.memset(extr_t[:], 0.001)
                    else:
                        nc.gpsimd.ap_gather(
                            extr_t[:], g_t[:],
                            eidx_t[:, tp * (DPX // 16):(tp + 1) * (DPX // 16)],
                            channels=P, num_elems=T, d=1, num_idxs=DPX)
                if (i >= 1 and not order2) or (order2 and i < NL):
                    if not order2:
                        tp = tiles[i - 1]
                    diff_t = dp.tile([P, DPX - 1], F32, tag="diff")
                    nc.vector.tensor_sub(diff_t[:], extr_t[:, 1:DPX],
                                         extr_t[:, 0:DPX - 1])
                    ps_t = pp.tile([B if mm8 else P, DPX - 1], F32, tag="ps")
                    nc.tensor.matmul(ps_t[:, 0:512], mmat_t[:],
                                     diff_t[:, 0:512], start=True, stop=True)
                    if dts[tp] > 512:
                        nc.tensor.matmul(ps_t[:, 512:DPX - 1], mmat_t[:],
                                         diff_t[:, 512:DPX - 1],
                                         start=True, stop=True)
                    st_t = sp.tile([B, DPX - 1], F32, tag="st")
                    nc.vector.tensor_copy(st_t[:, 0:dts[tp]],
                                          ps_t[0:B, 0:dts[tp]])
                    od = nc.sync.dma_start(
                        total_d[:, offs[tp]:offs[tp] + dts[tp]],
                        st_t[:, 0:dts[tp]])
                    out_dmas.append(od)
                    if prev_state["readback"] is not None:
                        _dep(od, prev_state["readback"],
                             "WAR total_d across steps")
                if i < NL:
                    cur = {"g": g_t[:]}

            # Epilogue
            if "noepi" in ab:
                prev_state["readback"] = None
                return
            tot_t = slp.tile([P, B * PB], F32, tag="tot")
            rb = nc.sync.dma_start(
                tot_t[:].rearrange("p (b c) -> p b c", b=B),
                total_d[:].rearrange("b (p c) -> p b c", p=P))
            for od in out_dmas:
                _dep(rb, od, "RAW total_d")
            nc.vector.tensor_add(tot_t[:], tot_t[:], bias_f[:])
            nc.scalar.activation(tot_t[:], tot_t[:],
                                 mybir.ActivationFunctionType.Tanh)
            nc.vector.tensor_add(aslice_t[:], aslice_t[:], tot_t[:])
            prev_state["readback"] = rb
            if step == steps - 1:
                return  # nothing consumes the collective after the last step
            if "nocc" in ab:
                return
            wb = nc.sync.dma_start(
                ag_in[:].rearrange("b (p c) -> p b c", p=P),
                aslice_t[:].rearrange("p (b c) -> p b c", b=B))
            if prev_state["collective"] is not None:
                _dep(wb, prev_state["collective"], "WAR ag_in")
            cc = nc.gpsimd.collective_compute(
                "AllGather", mybir.AluOpType.bypass,
                replica_groups=[list(range(NCD))],
                ins=[ag_in[:]], outs=[ag_out[:]])
            _dep(cc, wb, "RAW ag_in")
            if refresh3d:
                tr = nc.sync.dma_start(
                    table_t[:].rearrange("(k s) c -> k s c", s=16)[:, 0:B, :],
                    ag_out[:].rearrange("(k b) c -> k b c", k=NK)[:, :, 0:CH])
                _dep(tr, cc, "RAW ag_out")
            else:
                for k in range(NK):
                    tr = nc.sync.dma_start(
                        table_t[16 * k:16 * k + B, :],
                        ag_out[B * k:B * (k + 1), 0:CH])
                    _dep(tr, cc, "RAW ag_out")
            prev_state["collective"] = cc

        # Tiles needed by the final output (dests [tail0_lo, CH), where
        # tail0_lo = 117*PB rounds the 1024-dest output tail down to the
        # partition holding its first element).
        tail_lo = (CH - OUT_SIZE) // PB * PB
        tail_tiles = [t for t in range(NT) if offs[t] + dts[t] > tail_lo]
        for s in range(steps):
            if s == steps - 1 and s > 0:
                step_body(s, tiles=tail_tiles)
            else:
                step_body(s)

        # Final output from aslice_t (tail0 = 11476 = 117*98 + 10).
        nc.sync.dma_start(
            out_d[:, 0:88].rearrange("b (o c) -> o b c", o=1),
            aslice_t[117:118, :].rearrange("p (b c) -> p b c", b=B)[:, :, 10:98])
        nc.sync.dma_start(
            out_d[:, 88:970].rearrange("b (p c) -> p b c", p=9),
            aslice_t[118:127, :].rearrange("p (b c) -> p b c", b=B))
        nc.sync.dma_start(
            out_d[:, 970:1024].rearrange("b (o c) -> o b c", o=1),
            aslice_t[127:128, :].rearrange("p (b c) -> p b c", b=B)[:, :, 0:54])

    nc.compile()
    return nc


def _run(inputs_np, steps=STEPS):
    x = np.asarray(inputs_np["input_data"], np.float32)
    w = np.asarray(inputs_np["weights"], np.float32)
    bias = np.asarray(inputs_np["biases"], np.float32)
    f = np.asarray(inputs_np["from_idx"], np.int32)
    t_ = np.asarray(inputs_np["to_idx"], np.int32)
    in_maps, NT, dts, offs = _preprocess(x, w, bias, f, t_)
    nc = _build(NT, dts, offs, steps, scan_bypass=SCAN_BYPASS, mm8=MM8, refresh3d=REFRESH3D)
    res = bass_utils.run_bass_kernel_spmd(nc, in_maps, list(range(NCD)))
    return np.asarray(res.results[NCD - 1]["out"]).astype(np.float32)


def kernel(**inputs):
    return _run(inputs)
